# revision 9
# baseline (speedup 1.0000x reference)
"""BertSelfAttention on 8 Trainium2 NeuronCores (Bass/Tile).

Sharding: data-parallel over batch (B=2) x tensor-parallel over heads
(16 heads -> 4 groups of 4). Core c handles batch c//4, head group c%4,
holding column shards of Wq/Wk/Wv. No collectives.

v2 design (vs f32r baseline at 251us):
  * All matmul operands are bf16 (host-pre-cast; fp32/f32r moving operands
    stream at 2 cycles/column, bf16 at 1 -> 2x PE throughput; also halves
    input DMA bytes). PSUM accumulation stays fp32.
  * ScalarE does nothing but the 16.7M-element exp (its hard floor,
    ~1ns/elem/lane); all PSUM evacuations moved to VectorE
    (tensor_scalar_add applies the QK bias during evacuation).
  * Attention runs as 8 passes of (head-pair p, 512-wide q chunk qc);
    per kt: one [128,1024] score PSUM tile (2 row-packed 64-contraction
    matmuls, concurrent via PE row tiling), one [128,1024] exp ACT, two
    [65,512] ctx accumulations. Ctx matmuls lag one kt so the PE never
    waits on the exp it just requested.
  * V projection (pass 0) and the m=1 QK projection (passes 1-2, 2 MMs
    per kt) are interleaved into the attention loop, filling the PE's
    slack while ScalarE stays exp-saturated.
  * PSUM budget: scores 2x2 banks + ctx 2x1 + interleaved-proj 2x1 = 8.

Per-core layout (S=2048, 4 heads, d=64):
  xT    [1024, 2048] bf16   hidden states, this batch, pretransposed
  QT/KT [128, 2, 2048] bf16 d' on partitions (m in {0,1} = head pair)
  V     [2048, 260] bf16    head-major, 65 cols/head: 64 V dims + ones
                            column -> softmax denominators ride along
  out_raw [260, 2048] f32   4 heads x (64 ctx rows + 1 sums row)

Host unshards: out[b, :, g*256 + 64h + r] = (ctx_h / sums_h).T
"""

import sys

sys.path.insert(0, "/opt/trn_rl_repo")

import ml_dtypes
import numpy as np

import concourse.bass as bass
import concourse.mybir as mybir
import concourse.tile as tile
from concourse import bacc
from concourse import bass_utils as _bass_utils
from concourse.bass_utils import run_bass_kernel_spmd

# NOTE: the walrus --enable-ldw-opt=true rewrite (used by the f32r baseline)
# rejects bf16 LDWEIGHTS ("InstLdweights is not compatible with LDW
# optimization" — bf16 triggers the FWL weight-load path). Stock flags; the
# PE's 64-deep reorder window still pulls LDWEIGHTS ahead in silicon.

F32 = mybir.dt.float32
BF16 = mybir.dt.bfloat16
NP_BF16 = ml_dtypes.bfloat16

HIDDEN = 1024
NUM_HEADS = 16
HEAD = 64
B, S = 2, 2048
N_CORES = 8
GROUPS = 4                      # head groups (tensor parallel)
HG = NUM_HEADS // GROUPS        # heads per group = 4
DG = HG * HEAD                  # 256 cols per group
KT_TILES = HIDDEN // 128        # 8 contraction tiles for projections
ST_TILES = S // 128             # 16 sequence tiles
QC = 512                        # q chunk width
N_QC = S // QC                  # 4
VAUG = HG * (HEAD + 1)          # 260: [V_h | ones] per head


def _build_kernel():
    nc = bacc.Bacc("TRN2")

    xT = nc.dram_tensor("xT", [HIDDEN, S], BF16, kind="ExternalInput")
    # host pre-shuffled to SBUF layout [p, ko, wq128|wk128] per head pair m
    # (contiguous per partition -> cheap DMA descriptor generation)
    wqk0 = nc.dram_tensor("wqk0", [128, KT_TILES, 2 * 128], BF16,
                          kind="ExternalInput")
    wqk1 = nc.dram_tensor("wqk1", [128, KT_TILES, 2 * 128], BF16,
                          kind="ExternalInput")
    # wv pre-augmented (64 cols + zero col per head) and pre-shuffled to
    # [p, ko, 260]
    wv = nc.dram_tensor("wv", [128, KT_TILES, VAUG], BF16,
                        kind="ExternalInput")
    # per-partition bias cols: bq_m0, bq_m1, bk_m0, bk_m1
    bqk = nc.dram_tensor("bqk", [128, 4], F32, kind="ExternalInput")
    # bv (+1.0 at each head's ones column), host-replicated to all
    # partitions so DVE can apply it elementwise during V evacuation
    bv_aug = nc.dram_tensor("bv_aug", [128, VAUG], BF16, kind="ExternalInput")
    out_raw = nc.dram_tensor("out_raw", [VAUG, S], F32, kind="ExternalOutput")

    with tile.TileContext(nc) as tc:
        with (
            tc.tile_pool(name="consts", bufs=1) as consts,
            tc.tile_pool(name="work", bufs=4) as work,
            tc.tile_pool(name="outp", bufs=4) as outp,
            # scores [128,1024] double-buffered: 4 banks
            tc.tile_pool(name="ps2", bufs=2, space="PSUM") as ps2,
            # ctx accumulators [65,512] x2 per pass: 2 banks
            tc.tile_pool(name="psA", bufs=2, space="PSUM") as psA,
            # V-projection scratch: 1 bank
            tc.tile_pool(name="psV", bufs=1, space="PSUM") as psV,
            # QK-projection chunk accumulator: 1 bank
            tc.tile_pool(name="psK", bufs=1, space="PSUM") as psK,
        ):
            # ---- PE warmup: ~4us of dummy matmuls so the HAM clock
            # gate reaches 8/8 before the real projections start ----
            wu = consts.tile([128, 128], BF16)
            nc.vector.memset(wu[:], 0)
            wups = psV.tile([128, QC], F32, tag="v", name="warmup")
            N_WARM = 44
            for i in range(N_WARM):
                c = (i % 4) * 128
                nc.tensor.matmul(
                    wups[:, c:c + 128], wu[:], wu[:],
                    start=(i < 4), stop=(i >= N_WARM - 4),
                )

            # ---- input DMAs: xT split into (sc, ko) pieces, SEQ-major so
            # attention over early q/k blocks can start while the rest of
            # x streams in; triggers spread over sync/scalar/gpsimd ----
            wqk_sb = consts.tile([128, 2, KT_TILES, 2 * 128], BF16)
            xT_sb = consts.tile([128, KT_TILES, S], BF16)
            xT_r = xT.rearrange("(ko p) s -> p ko s", p=128)
            wv_sb = consts.tile([128, KT_TILES, VAUG], BF16)
            bqk_sb = consts.tile([128, 4], F32)
            bvaug_sb = consts.tile([128, VAUG], BF16)

            def xt_piece(sc, ko):
                sl = (slice(None), ko, slice(sc * QC, (sc + 1) * QC))
                return (xT_sb[sl], xT_r[sl])

            queues = {"sync": [], "scalar": [], "gpsimd": []}
            qnames = ["sync", "scalar", "gpsimd"]
            queues["sync"].append((wqk_sb[:, 0], wqk0[:]))
            queues["sync"].append((bqk_sb[:], bqk[:]))
            for i, (sc, ko) in enumerate(
                [(sc, ko) for sc in range(N_QC) for ko in range(KT_TILES)]
            ):
                queues[qnames[i % 3]].append(xt_piece(sc, ko))
                if sc == 0 and ko == 5:
                    queues["gpsimd"].append((wv_sb[:], wv[:]))
                    queues["scalar"].append((bvaug_sb[:], bv_aug[:]))
                if sc == 1 and ko == 7:
                    queues["gpsimd"].append((wqk_sb[:, 1], wqk1[:]))
            for qn, lst in queues.items():
                eng = getattr(nc, qn)
                for dst, sr in lst:
                    eng.dma_start(dst, sr)

            QT_sb = consts.tile([128, 2, S], BF16)
            KT_sb = consts.tile([128, 2, S], BF16)
            v_sb = consts.tile([128, ST_TILES, VAUG], BF16)

            def w_slice(w, m, ko):
                return wqk_sb[:, m, ko, w * 128:(w + 1) * 128]

            def qk_evac(dst, ps, w, m, sc):
                # bias[d'] is per-partition: DVE adds it during evacuation
                nc.vector.tensor_scalar_add(
                    dst[:, m, sc * QC:(sc + 1) * QC],
                    ps,
                    bqk_sb[:, w * 2 + m:w * 2 + m + 1],
                )

            # QK projection as chunks of 8 contraction matmuls; drained
            # 2 per attention kt-iteration against per-pass deadlines
            def make_chunk(w, m, sc):
                state = {}

                def mk(ko):
                    def thunk():
                        if ko == 0:
                            state["ps"] = psK.tile(
                                [128, QC], F32, tag="k",
                                name=f"pj{w}{m}{sc}",
                            )
                        nc.tensor.matmul(
                            state["ps"][:], w_slice(w, m, ko),
                            xT_sb[:, ko, sc * QC:(sc + 1) * QC],
                            start=(ko == 0), stop=(ko == KT_TILES - 1),
                        )
                        if ko == KT_TILES - 1:
                            qk_evac(
                                QT_sb if w == 0 else KT_sb,
                                state["ps"][:], w, m, sc,
                            )
                    return thunk
                return [mk(ko) for ko in range(KT_TILES)]

            Q, K = 0, 1
            # per-pass proj thunk queues (2 drained per kt):
            #   lead: Q/K(m0,sc0)   (before pass 0)
            #   pass0: K(m0,sc1..3) just ahead of the kt that needs them,
            #          then Q(m0,sc1); v-projection also rides here
            #   pass1: Q(m0,sc2), Q(m0,sc3), Q(m1,sc0), K(m1,sc0)
            #   pass2: Q/K(m1,sc1), Q/K(m1,sc2)
            #   pass3: Q(m1,sc3), K(m1,sc3)
            lead_chunks = make_chunk(Q, 0, 0) + make_chunk(K, 0, 0)
            pass_proj = {
                0: (make_chunk(K, 0, 1) + make_chunk(K, 0, 2)
                    + make_chunk(K, 0, 3) + make_chunk(Q, 0, 1)),
                1: (make_chunk(Q, 0, 2) + make_chunk(Q, 0, 3)
                    + make_chunk(Q, 1, 0) + make_chunk(K, 1, 0)),
                2: (make_chunk(Q, 1, 1) + make_chunk(K, 1, 1)
                    + make_chunk(Q, 1, 2) + make_chunk(K, 1, 2)),
                3: make_chunk(Q, 1, 3) + make_chunk(K, 1, 3),
            }
            for t in lead_chunks:
                t()

            def v_proj(st):
                psv = psV.tile([128, QC], F32, tag="v", name="ps_v")
                for ko in range(KT_TILES):
                    nc.tensor.matmul(
                        psv[:, 0:VAUG],
                        xT_sb[:, ko, st * 128:(st + 1) * 128],
                        wv_sb[:, ko, :],
                        start=(ko == 0), stop=(ko == KT_TILES - 1),
                    )
                # bias + per-head ones columns land during evacuation
                nc.vector.tensor_add(
                    v_sb[:, st, :], psv[:, 0:VAUG], bvaug_sb[:]
                )

            def attn_pass(pi, p, qc):
                proj = pass_proj.get(pi, [])
                ctxs = [
                    psA.tile([65, QC], F32, tag="sA", name=f"ctx{hh}")
                    for hh in range(2)
                ]

                def ctx_mms(es, kt):
                    for hh in range(2):
                        h = 2 * p + hh
                        nc.tensor.matmul(
                            ctxs[hh][:],
                            v_sb[:, kt, h * 65:(h + 1) * 65],
                            es[:, hh * QC:(hh + 1) * QC],
                            start=(kt == 0), stop=(kt == ST_TILES - 1),
                        )

                prev = None
                for kt in range(ST_TILES):
                    if pi == 0:
                        v_proj(kt)  # ctx at kt consumes exactly V tile kt
                    for t in proj[kt * 2:kt * 2 + 2]:
                        t()
                    ssc = ps2.tile([128, 1024], F32, tag="s2", name="ssc")
                    for hh in range(2):  # row-packed pair, runs concurrent
                        rows = slice(hh * 64, hh * 64 + 64)
                        nc.tensor.matmul(
                            ssc[:, hh * QC:(hh + 1) * QC],
                            KT_sb[rows, p, kt * 128:(kt + 1) * 128],
                            QT_sb[rows, p, qc * QC:(qc + 1) * QC],
                            start=True, stop=True,
                        )
                    es = work.tile([128, 1024], BF16, tag="es", name="es")
                    nc.scalar.activation(
                        es[:], ssc[:],
                        mybir.ActivationFunctionType.Exp,
                        scale=0.125,
                    )
                    if prev is not None:
                        ctx_mms(*prev)  # lag 1 kt: never stall on fresh exp
                    prev = (es, kt)
                ctx_mms(*prev)
                for hh in range(2):
                    h = 2 * p + hh
                    ctx_sb = outp.tile([65, QC], F32, tag="o", name="ctx_sb")
                    nc.vector.tensor_copy(out=ctx_sb[:], in_=ctxs[hh][:])
                    nc.sync.dma_start(
                        out_raw[h * 65:(h + 1) * 65, qc * QC:(qc + 1) * QC],
                        ctx_sb[:],
                    )

            for pi, (p, qc) in enumerate(
                [(0, 0), (0, 1), (0, 2), (0, 3),
                 (1, 0), (1, 1), (1, 2), (1, 3)]
            ):
                attn_pass(pi, p, qc)
    nc.compile()
    return nc


_NC_CACHE = None


def _get_nc():
    global _NC_CACHE
    if _NC_CACHE is None:
        _NC_CACHE = _build_kernel()
    return _NC_CACHE


def _prep_core_inputs(hidden_states, Wq, bq, Wk, bk, Wv, bv):
    """Host-side sharding: returns list of 8 in_maps (bf16 pre-cast)."""
    xTs = [
        np.ascontiguousarray(hidden_states[b].T).astype(NP_BF16)
        for b in range(B)
    ]
    in_maps = []
    for c in range(N_CORES):
        b, g = divmod(c, GROUPS)
        cs = slice(g * DG, (g + 1) * DG)
        wq_g, wk_g, wv_g = Wq[:, cs], Wk[:, cs], Wv[:, cs]
        bq_g, bk_g, bv_g = bq[cs], bk[cs], bv[cs]

        # per m: [p, ko, wq 128 | wk 128] (SBUF layout, contiguous)
        wq_r = wq_g.reshape(KT_TILES, 128, DG).transpose(1, 0, 2)
        wk_r = wk_g.reshape(KT_TILES, 128, DG).transpose(1, 0, 2)
        wqk_ms = [
            np.concatenate(
                [wq_r[:, :, m * 128:(m + 1) * 128],
                 wk_r[:, :, m * 128:(m + 1) * 128]], axis=2,
            )
            for m in range(2)
        ]

        wv_aug = np.zeros((HIDDEN, VAUG), dtype=np.float32)
        bv_aug = np.zeros((1, VAUG), dtype=np.float32)  # replicated below
        for h in range(HG):
            wv_aug[:, h * 65:h * 65 + 64] = wv_g[:, h * 64:(h + 1) * 64]
            bv_aug[0, h * 65:h * 65 + 64] = bv_g[h * 64:(h + 1) * 64]
            bv_aug[0, h * 65 + 64] = 1.0

        bqk = np.stack(
            [bq_g[:128], bq_g[128:], bk_g[:128], bk_g[128:]], axis=1
        ).astype(np.float32)

        wv_r = wv_aug.reshape(KT_TILES, 128, VAUG).transpose(1, 0, 2)
        in_maps.append(
            {
                "xT": xTs[b],
                "wqk0": np.ascontiguousarray(wqk_ms[0]).astype(NP_BF16),
                "wqk1": np.ascontiguousarray(wqk_ms[1]).astype(NP_BF16),
                "wv": np.ascontiguousarray(wv_r).astype(NP_BF16),
                "bqk": np.ascontiguousarray(bqk),
                "bv_aug": np.broadcast_to(
                    bv_aug.astype(NP_BF16), (128, VAUG)
                ).copy(),
            }
        )
    return in_maps


def _unshard(results):
    out = np.empty((B, S, HIDDEN), dtype=np.float32)
    for c in range(N_CORES):
        b, g = divmod(c, GROUPS)
        raw = results[c]["out_raw"]  # [260, 2048]
        for h in range(HG):
            ctx = raw[h * 65:h * 65 + 64]          # [64, S]
            sums = raw[h * 65 + 64]                # [S]
            col0 = g * DG + h * HEAD
            out[b, :, col0:col0 + HEAD] = (ctx / sums).T
    return out


def kernel(**inputs):
    inputs = {k: np.asarray(v, dtype=np.float32) for k, v in inputs.items()}
    nc = _get_nc()
    in_maps = _prep_core_inputs(**inputs)
    res = run_bass_kernel_spmd(nc, in_maps, core_ids=list(range(N_CORES)))
    return _unshard(res.results)


if __name__ == "__main__":
    rng = np.random.default_rng(0)
    scale = 1.0 / np.sqrt(HIDDEN)
    ins = {
        "hidden_states": rng.standard_normal((B, S, HIDDEN), dtype=np.float32),
        "Wq": rng.standard_normal((HIDDEN, HIDDEN), dtype=np.float32) * scale,
        "bq": rng.standard_normal(HIDDEN, dtype=np.float32) * 0.01,
        "Wk": rng.standard_normal((HIDDEN, HIDDEN), dtype=np.float32) * scale,
        "bk": rng.standard_normal(HIDDEN, dtype=np.float32) * 0.01,
        "Wv": rng.standard_normal((HIDDEN, HIDDEN), dtype=np.float32) * scale,
        "bv": rng.standard_normal(HIDDEN, dtype=np.float32) * 0.01,
    }
    out = kernel(**ins)

    def ref(x, Wq, bq, Wk, bk, Wv, bv):
        q = (x @ Wq + bq).reshape(B, S, NUM_HEADS, HEAD).transpose(0, 2, 1, 3)
        k = (x @ Wk + bk).reshape(B, S, NUM_HEADS, HEAD).transpose(0, 2, 1, 3)
        v = (x @ Wv + bv).reshape(B, S, NUM_HEADS, HEAD).transpose(0, 2, 1, 3)
        s = np.einsum("bhqd,bhkd->bhqk", q, k) / np.sqrt(HEAD)
        s = s - s.max(-1, keepdims=True)
        p = np.exp(s)
        p /= p.sum(-1, keepdims=True)
        c = np.einsum("bhqk,bhkd->bhqd", p, v)
        return c.transpose(0, 2, 1, 3).reshape(B, S, HIDDEN)

    exp = ref(
        ins["hidden_states"].astype(np.float64),
        ins["Wq"].astype(np.float64), ins["bq"].astype(np.float64),
        ins["Wk"].astype(np.float64), ins["bk"].astype(np.float64),
        ins["Wv"].astype(np.float64), ins["bv"].astype(np.float64),
    )
    print("L2 rel err:", np.linalg.norm(out - exp) / np.linalg.norm(exp))
    print("max abs err:", np.abs(out - exp).max())


# revision 12
# speedup vs baseline: 1.0267x; 1.0267x over previous
"""BertSelfAttention on 8 Trainium2 NeuronCores (Bass/Tile).

Sharding: data-parallel over batch (B=2) x tensor-parallel over heads
(16 heads -> 4 groups of 4). Core c handles batch c//4, head group c%4,
holding column shards of Wq/Wk/Wv. No collectives.

v2 design (vs f32r baseline at 251us):
  * All matmul operands are bf16 (host-pre-cast; fp32/f32r moving operands
    stream at 2 cycles/column, bf16 at 1 -> 2x PE throughput; also halves
    input DMA bytes). PSUM accumulation stays fp32.
  * ScalarE does nothing but the 16.7M-element exp (its hard floor,
    ~1ns/elem/lane); all PSUM evacuations moved to VectorE
    (tensor_scalar_add applies the QK bias during evacuation).
  * Attention runs as 8 passes of (head-pair p, 512-wide q chunk qc);
    per kt: one [128,1024] score PSUM tile (2 row-packed 64-contraction
    matmuls, concurrent via PE row tiling), one [128,1024] exp ACT, two
    [65,512] ctx accumulations. Ctx matmuls lag one kt so the PE never
    waits on the exp it just requested.
  * V projection (pass 0) and the m=1 QK projection (passes 1-2, 2 MMs
    per kt) are interleaved into the attention loop, filling the PE's
    slack while ScalarE stays exp-saturated.
  * PSUM budget: scores 2x2 banks + ctx 2x1 + interleaved-proj 2x1 = 8.

Per-core layout (S=2048, 4 heads, d=64):
  xT    [1024, 2048] bf16   hidden states, this batch, pretransposed
  QT/KT [128, 2, 2048] bf16 d' on partitions (m in {0,1} = head pair)
  V     [2048, 260] bf16    head-major, 65 cols/head: 64 V dims + ones
                            column -> softmax denominators ride along
  out_raw [260, 2048] f32   4 heads x (64 ctx rows + 1 sums row)

Host unshards: out[b, :, g*256 + 64h + r] = (ctx_h / sums_h).T
"""

import sys

sys.path.insert(0, "/opt/trn_rl_repo")

import ml_dtypes
import numpy as np

import concourse.bass as bass
import concourse.mybir as mybir
import concourse.tile as tile
from concourse import bacc
from concourse import bass_utils as _bass_utils
from concourse.bass_utils import run_bass_kernel_spmd

# NOTE: the walrus --enable-ldw-opt=true rewrite (used by the f32r baseline)
# rejects bf16 LDWEIGHTS ("InstLdweights is not compatible with LDW
# optimization" — bf16 triggers the FWL weight-load path). Stock flags; the
# PE's 64-deep reorder window still pulls LDWEIGHTS ahead in silicon.

F32 = mybir.dt.float32
BF16 = mybir.dt.bfloat16
NP_BF16 = ml_dtypes.bfloat16

HIDDEN = 1024
NUM_HEADS = 16
HEAD = 64
B, S = 2, 2048
N_CORES = 8
GROUPS = 4                      # head groups (tensor parallel)
HG = NUM_HEADS // GROUPS        # heads per group = 4
DG = HG * HEAD                  # 256 cols per group
KT_TILES = HIDDEN // 128        # 8 contraction tiles for projections
ST_TILES = S // 128             # 16 sequence tiles
QC = 512                        # q chunk width
N_QC = S // QC                  # 4
VAUG = HG * (HEAD + 1)          # 260: [V_h | ones] per head


def _build_kernel():
    nc = bacc.Bacc("TRN2")

    xT = nc.dram_tensor("xT", [HIDDEN, S], BF16, kind="ExternalInput")
    # host pre-shuffled to SBUF layout [p, ko, wq128|wk128] per head pair m
    # (contiguous per partition -> cheap DMA descriptor generation)
    wqk0 = nc.dram_tensor("wqk0", [128, KT_TILES, 2 * 128], BF16,
                          kind="ExternalInput")
    wqk1 = nc.dram_tensor("wqk1", [128, KT_TILES, 2 * 128], BF16,
                          kind="ExternalInput")
    # wv pre-augmented (64 cols + zero col per head) and pre-shuffled to
    # [p, ko, 260]
    wv = nc.dram_tensor("wv", [128, KT_TILES, VAUG], BF16,
                        kind="ExternalInput")
    # per-partition bias cols: bq_m0, bq_m1, bk_m0, bk_m1
    bqk = nc.dram_tensor("bqk", [128, 4], F32, kind="ExternalInput")
    # bv (+1.0 at each head's ones column), host-replicated to all
    # partitions so DVE can apply it elementwise during V evacuation
    bv_aug = nc.dram_tensor("bv_aug", [128, VAUG], BF16, kind="ExternalInput")
    out_raw = nc.dram_tensor("out_raw", [VAUG, S], F32, kind="ExternalOutput")

    with tile.TileContext(nc) as tc:
        with (
            tc.tile_pool(name="consts", bufs=1) as consts,
            tc.tile_pool(name="work", bufs=4) as work,
            tc.tile_pool(name="outp", bufs=4) as outp,
            # scores [128,1024] double-buffered: 4 banks
            tc.tile_pool(name="ps2", bufs=2, space="PSUM") as ps2,
            # ctx accumulators [65,512] x2 per pass: 2 banks
            tc.tile_pool(name="psA", bufs=2, space="PSUM") as psA,
            # V-projection scratch: 1 bank
            tc.tile_pool(name="psV", bufs=1, space="PSUM") as psV,
            # QK-projection chunk accumulator: 1 bank
            tc.tile_pool(name="psK", bufs=1, space="PSUM") as psK,
        ):
            # ---- PE warmup: ~4us of dummy matmuls so the HAM clock
            # gate reaches 8/8 before the real projections start ----
            wu = consts.tile([128, 128], BF16)
            nc.vector.memset(wu[:], 0)
            wups = psV.tile([128, QC], F32, tag="v", name="warmup")
            N_WARM = 60
            for i in range(N_WARM):
                c = (i % 4) * 128
                nc.tensor.matmul(
                    wups[:, c:c + 128], wu[:], wu[:],
                    start=(i < 4), stop=(i >= N_WARM - 4),
                )

            # ---- input DMAs: xT split into (sc, ko) pieces, SEQ-major so
            # attention over early q/k blocks can start while the rest of
            # x streams in; triggers spread over sync/scalar/gpsimd ----
            wqk_sb = consts.tile([128, 2, KT_TILES, 2 * 128], BF16)
            xT_sb = consts.tile([128, KT_TILES, S], BF16)
            xT_r = xT.rearrange("(ko p) s -> p ko s", p=128)
            wv_sb = consts.tile([128, KT_TILES, VAUG], BF16)
            bqk_sb = consts.tile([128, 4], F32)
            bvaug_sb = consts.tile([128, VAUG], BF16)

            def xt_piece(sc, ko):
                sl = (slice(None), ko, slice(sc * QC, (sc + 1) * QC))
                return (xT_sb[sl], xT_r[sl])

            queues = {"sync": [], "scalar": [], "gpsimd": []}
            qnames = ["sync", "scalar", "gpsimd"]
            h = KT_TILES // 2
            queues["scalar"].append((wqk_sb[:, 0, 0:h], wqk0[:, 0:h]))
            queues["gpsimd"].append((wqk_sb[:, 0, h:], wqk0[:, h:]))
            for i, (sc, ko) in enumerate(
                [(sc, ko) for sc in range(N_QC) for ko in range(KT_TILES)]
            ):
                queues[qnames[i % 3]].append(xt_piece(sc, ko))
                if sc == 0 and ko == 5:
                    queues["gpsimd"].append((wv_sb[:], wv[:]))
                    queues["scalar"].append((bvaug_sb[:], bv_aug[:]))
                    queues["sync"].append((bqk_sb[:], bqk[:]))
                if sc == 1 and ko == 7:
                    queues["gpsimd"].append((wqk_sb[:, 1], wqk1[:]))
            for qn, lst in queues.items():
                eng = getattr(nc, qn)
                for dst, sr in lst:
                    eng.dma_start(dst, sr)

            QT_sb = consts.tile([128, 2, S], BF16)
            KT_sb = consts.tile([128, 2, S], BF16)
            v_sb = consts.tile([128, ST_TILES, VAUG], BF16)

            def w_slice(w, m, ko):
                return wqk_sb[:, m, ko, w * 128:(w + 1) * 128]

            def qk_evac(dst, ps, w, m, sc):
                # bias[d'] is per-partition: DVE adds it during evacuation
                nc.vector.tensor_scalar_add(
                    dst[:, m, sc * QC:(sc + 1) * QC],
                    ps,
                    bqk_sb[:, w * 2 + m:w * 2 + m + 1],
                )

            # QK projection as chunks of 8 contraction matmuls; drained
            # 2 per attention kt-iteration against per-pass deadlines
            def make_chunk(w, m, sc, pool=None):
                state = {}
                pl = pool if pool is not None else psK
                ptag = "v" if pool is not None else "k"

                def mk(ko):
                    def thunk():
                        if ko == 0:
                            state["ps"] = pl.tile(
                                [128, QC], F32, tag=ptag, name=f"pj{w}{m}{sc}",
                            )
                        nc.tensor.matmul(
                            state["ps"][:], w_slice(w, m, ko),
                            xT_sb[:, ko, sc * QC:(sc + 1) * QC],
                            start=(ko == 0), stop=(ko == KT_TILES - 1),
                        )
                        if ko == KT_TILES - 1:
                            qk_evac(
                                QT_sb if w == 0 else KT_sb,
                                state["ps"][:], w, m, sc,
                            )
                    return thunk
                return [mk(ko) for ko in range(KT_TILES)]

            Q, K = 0, 1
            # per-pass proj thunk queues (2 drained per kt):
            #   lead: Q/K(m0,sc0)   (before pass 0)
            #   pass0: K(m0,sc1..3) just ahead of the kt that needs them,
            #          then Q(m0,sc1); v-projection also rides here
            #   pass1: Q(m0,sc2), Q(m0,sc3), Q(m1,sc0), K(m1,sc0)
            #   pass2: Q/K(m1,sc1), Q/K(m1,sc2)
            #   pass3: Q(m1,sc3), K(m1,sc3)
            lead_chunks = make_chunk(Q, 0, 0) + make_chunk(K, 0, 0)
            pass_proj = {
                0: (make_chunk(K, 0, 1) + make_chunk(K, 0, 2)
                    + make_chunk(K, 0, 3) + make_chunk(Q, 0, 1)),
                1: (make_chunk(Q, 0, 2) + make_chunk(Q, 0, 3, psV)
                    + make_chunk(Q, 1, 0) + make_chunk(K, 1, 0, psV)),
                2: (make_chunk(Q, 1, 1) + make_chunk(K, 1, 1, psV)
                    + make_chunk(Q, 1, 2) + make_chunk(K, 1, 2, psV)),
                3: make_chunk(Q, 1, 3) + make_chunk(K, 1, 3, psV),
            }
            # drain the lead with dummy filler matmuls between real ones:
            # the real MMs are DMA-gated, the fillers keep the PE's HAM
            # activity window busy so the clock stays at 8/8
            for i, t in enumerate(lead_chunks):
                t()
                for j in range(2):
                    c = ((2 * i + j) % 4) * 128
                    nc.tensor.matmul(
                        wups[:, c:c + 128], wu[:], wu[:],
                        start=True, stop=True,
                    )

            def v_proj(st):
                psv = psV.tile([128, QC], F32, tag="v", name="ps_v")
                for ko in range(KT_TILES):
                    nc.tensor.matmul(
                        psv[:, 0:VAUG],
                        xT_sb[:, ko, st * 128:(st + 1) * 128],
                        wv_sb[:, ko, :],
                        start=(ko == 0), stop=(ko == KT_TILES - 1),
                    )
                # bias + per-head ones columns land during evacuation
                nc.vector.tensor_add(
                    v_sb[:, st, :], psv[:, 0:VAUG], bvaug_sb[:]
                )

            def attn_pass(pi, p, qc):
                proj = pass_proj.get(pi, [])
                ctxs = [
                    psA.tile([65, QC], F32, tag="sA", name=f"ctx{hh}")
                    for hh in range(2)
                ]

                def ctx_mms(es, kt):
                    for hh in range(2):
                        h = 2 * p + hh
                        nc.tensor.matmul(
                            ctxs[hh][:],
                            v_sb[:, kt, h * 65:(h + 1) * 65],
                            es[:, hh * QC:(hh + 1) * QC],
                            start=(kt == 0), stop=(kt == ST_TILES - 1),
                        )

                prev = None
                for kt in range(ST_TILES):
                    if pi == 0:
                        v_proj(kt)  # ctx at kt consumes exactly V tile kt
                    for t in proj[kt * 2:kt * 2 + 2]:
                        t()
                    ssc = ps2.tile([128, 1024], F32, tag="s2", name="ssc")
                    for hh in range(2):  # row-packed pair, runs concurrent
                        rows = slice(hh * 64, hh * 64 + 64)
                        nc.tensor.matmul(
                            ssc[:, hh * QC:(hh + 1) * QC],
                            KT_sb[rows, p, kt * 128:(kt + 1) * 128],
                            QT_sb[rows, p, qc * QC:(qc + 1) * QC],
                            start=True, stop=True,
                        )
                    es = work.tile([128, 1024], BF16, tag="es", name="es")
                    nc.scalar.activation(
                        es[:], ssc[:],
                        mybir.ActivationFunctionType.Exp,
                        scale=0.125,
                    )
                    if prev is not None:
                        ctx_mms(*prev)  # lag 1 kt: never stall on fresh exp
                    prev = (es, kt)
                ctx_mms(*prev)
                for hh in range(2):
                    h = 2 * p + hh
                    ctx_sb = outp.tile([65, QC], F32, tag="o", name="ctx_sb")
                    nc.vector.tensor_copy(out=ctx_sb[:], in_=ctxs[hh][:])
                    nc.sync.dma_start(
                        out_raw[h * 65:(h + 1) * 65, qc * QC:(qc + 1) * QC],
                        ctx_sb[:],
                    )

            for pi, (p, qc) in enumerate(
                [(0, 0), (0, 1), (0, 2), (0, 3),
                 (1, 0), (1, 1), (1, 2), (1, 3)]
            ):
                attn_pass(pi, p, qc)
    nc.compile()
    return nc


_NC_CACHE = None


def _get_nc():
    global _NC_CACHE
    if _NC_CACHE is None:
        _NC_CACHE = _build_kernel()
    return _NC_CACHE


def _prep_core_inputs(hidden_states, Wq, bq, Wk, bk, Wv, bv):
    """Host-side sharding: returns list of 8 in_maps (bf16 pre-cast)."""
    xTs = [
        np.ascontiguousarray(hidden_states[b].T).astype(NP_BF16)
        for b in range(B)
    ]
    in_maps = []
    for c in range(N_CORES):
        b, g = divmod(c, GROUPS)
        cs = slice(g * DG, (g + 1) * DG)
        wq_g, wk_g, wv_g = Wq[:, cs], Wk[:, cs], Wv[:, cs]
        bq_g, bk_g, bv_g = bq[cs], bk[cs], bv[cs]

        # per m: [p, ko, wq 128 | wk 128] (SBUF layout, contiguous)
        wq_r = wq_g.reshape(KT_TILES, 128, DG).transpose(1, 0, 2)
        wk_r = wk_g.reshape(KT_TILES, 128, DG).transpose(1, 0, 2)
        wqk_ms = [
            np.concatenate(
                [wq_r[:, :, m * 128:(m + 1) * 128],
                 wk_r[:, :, m * 128:(m + 1) * 128]], axis=2,
            )
            for m in range(2)
        ]

        wv_aug = np.zeros((HIDDEN, VAUG), dtype=np.float32)
        bv_aug = np.zeros((1, VAUG), dtype=np.float32)  # replicated below
        for h in range(HG):
            wv_aug[:, h * 65:h * 65 + 64] = wv_g[:, h * 64:(h + 1) * 64]
            bv_aug[0, h * 65:h * 65 + 64] = bv_g[h * 64:(h + 1) * 64]
            bv_aug[0, h * 65 + 64] = 1.0

        bqk = np.stack(
            [bq_g[:128], bq_g[128:], bk_g[:128], bk_g[128:]], axis=1
        ).astype(np.float32)

        wv_r = wv_aug.reshape(KT_TILES, 128, VAUG).transpose(1, 0, 2)
        in_maps.append(
            {
                "xT": xTs[b],
                "wqk0": np.ascontiguousarray(wqk_ms[0]).astype(NP_BF16),
                "wqk1": np.ascontiguousarray(wqk_ms[1]).astype(NP_BF16),
                "wv": np.ascontiguousarray(wv_r).astype(NP_BF16),
                "bqk": np.ascontiguousarray(bqk),
                "bv_aug": np.broadcast_to(
                    bv_aug.astype(NP_BF16), (128, VAUG)
                ).copy(),
            }
        )
    return in_maps


def _unshard(results):
    out = np.empty((B, S, HIDDEN), dtype=np.float32)
    for c in range(N_CORES):
        b, g = divmod(c, GROUPS)
        raw = results[c]["out_raw"]  # [260, 2048]
        for h in range(HG):
            ctx = raw[h * 65:h * 65 + 64]          # [64, S]
            sums = raw[h * 65 + 64]                # [S]
            col0 = g * DG + h * HEAD
            out[b, :, col0:col0 + HEAD] = (ctx / sums).T
    return out


def kernel(**inputs):
    inputs = {k: np.asarray(v, dtype=np.float32) for k, v in inputs.items()}
    nc = _get_nc()
    in_maps = _prep_core_inputs(**inputs)
    res = run_bass_kernel_spmd(nc, in_maps, core_ids=list(range(N_CORES)))
    return _unshard(res.results)


if __name__ == "__main__":
    rng = np.random.default_rng(0)
    scale = 1.0 / np.sqrt(HIDDEN)
    ins = {
        "hidden_states": rng.standard_normal((B, S, HIDDEN), dtype=np.float32),
        "Wq": rng.standard_normal((HIDDEN, HIDDEN), dtype=np.float32) * scale,
        "bq": rng.standard_normal(HIDDEN, dtype=np.float32) * 0.01,
        "Wk": rng.standard_normal((HIDDEN, HIDDEN), dtype=np.float32) * scale,
        "bk": rng.standard_normal(HIDDEN, dtype=np.float32) * 0.01,
        "Wv": rng.standard_normal((HIDDEN, HIDDEN), dtype=np.float32) * scale,
        "bv": rng.standard_normal(HIDDEN, dtype=np.float32) * 0.01,
    }
    out = kernel(**ins)

    def ref(x, Wq, bq, Wk, bk, Wv, bv):
        q = (x @ Wq + bq).reshape(B, S, NUM_HEADS, HEAD).transpose(0, 2, 1, 3)
        k = (x @ Wk + bk).reshape(B, S, NUM_HEADS, HEAD).transpose(0, 2, 1, 3)
        v = (x @ Wv + bv).reshape(B, S, NUM_HEADS, HEAD).transpose(0, 2, 1, 3)
        s = np.einsum("bhqd,bhkd->bhqk", q, k) / np.sqrt(HEAD)
        s = s - s.max(-1, keepdims=True)
        p = np.exp(s)
        p /= p.sum(-1, keepdims=True)
        c = np.einsum("bhqk,bhkd->bhqd", p, v)
        return c.transpose(0, 2, 1, 3).reshape(B, S, HIDDEN)

    exp = ref(
        ins["hidden_states"].astype(np.float64),
        ins["Wq"].astype(np.float64), ins["bq"].astype(np.float64),
        ins["Wk"].astype(np.float64), ins["bk"].astype(np.float64),
        ins["Wv"].astype(np.float64), ins["bv"].astype(np.float64),
    )
    print("L2 rel err:", np.linalg.norm(out - exp) / np.linalg.norm(exp))
    print("max abs err:", np.abs(out - exp).max())


# revision 13
# speedup vs baseline: 1.0293x; 1.0026x over previous
"""BertSelfAttention on 8 Trainium2 NeuronCores (Bass/Tile).

Sharding: data-parallel over batch (B=2) x tensor-parallel over heads
(16 heads -> 4 groups of 4). Core c handles batch c//4, head group c%4,
holding column shards of Wq/Wk/Wv. No collectives.

v2 design (vs f32r baseline at 251us):
  * All matmul operands are bf16 (host-pre-cast; fp32/f32r moving operands
    stream at 2 cycles/column, bf16 at 1 -> 2x PE throughput; also halves
    input DMA bytes). PSUM accumulation stays fp32.
  * ScalarE does nothing but the 16.7M-element exp (its hard floor,
    ~1ns/elem/lane); all PSUM evacuations moved to VectorE
    (tensor_scalar_add applies the QK bias during evacuation).
  * Attention runs as 8 passes of (head-pair p, 512-wide q chunk qc);
    per kt: one [128,1024] score PSUM tile (2 row-packed 64-contraction
    matmuls, concurrent via PE row tiling), one [128,1024] exp ACT, two
    [65,512] ctx accumulations. Ctx matmuls lag one kt so the PE never
    waits on the exp it just requested.
  * V projection (pass 0) and the m=1 QK projection (passes 1-2, 2 MMs
    per kt) are interleaved into the attention loop, filling the PE's
    slack while ScalarE stays exp-saturated.
  * PSUM budget: scores 2x2 banks + ctx 2x1 + interleaved-proj 2x1 = 8.

Per-core layout (S=2048, 4 heads, d=64):
  xT    [1024, 2048] bf16   hidden states, this batch, pretransposed
  QT/KT [128, 2, 2048] bf16 d' on partitions (m in {0,1} = head pair)
  V     [2048, 260] bf16    head-major, 65 cols/head: 64 V dims + ones
                            column -> softmax denominators ride along
  out_raw [260, 2048] f32   4 heads x (64 ctx rows + 1 sums row)

Host unshards: out[b, :, g*256 + 64h + r] = (ctx_h / sums_h).T
"""

import sys

sys.path.insert(0, "/opt/trn_rl_repo")

import ml_dtypes
import numpy as np

import concourse.bass as bass
import concourse.mybir as mybir
import concourse.tile as tile
from concourse import bacc
from concourse import bass_utils as _bass_utils
from concourse.bass_utils import run_bass_kernel_spmd

# NOTE: the walrus --enable-ldw-opt=true rewrite (used by the f32r baseline)
# rejects bf16 LDWEIGHTS ("InstLdweights is not compatible with LDW
# optimization" — bf16 triggers the FWL weight-load path). Stock flags; the
# PE's 64-deep reorder window still pulls LDWEIGHTS ahead in silicon.

F32 = mybir.dt.float32
BF16 = mybir.dt.bfloat16
NP_BF16 = ml_dtypes.bfloat16

HIDDEN = 1024
NUM_HEADS = 16
HEAD = 64
B, S = 2, 2048
N_CORES = 8
GROUPS = 4                      # head groups (tensor parallel)
HG = NUM_HEADS // GROUPS        # heads per group = 4
DG = HG * HEAD                  # 256 cols per group
KT_TILES = HIDDEN // 128        # 8 contraction tiles for projections
ST_TILES = S // 128             # 16 sequence tiles
QC = 512                        # q chunk width
N_QC = S // QC                  # 4
VAUG = HG * (HEAD + 1)          # 260: [V_h | ones] per head


def _build_kernel():
    nc = bacc.Bacc("TRN2")

    xT = nc.dram_tensor("xT", [HIDDEN, S], BF16, kind="ExternalInput")
    # host pre-shuffled to SBUF layout [p, ko, wq128|wk128] per head pair m
    # (contiguous per partition -> cheap DMA descriptor generation)
    wqk0 = nc.dram_tensor("wqk0", [128, KT_TILES, 2 * 128], BF16,
                          kind="ExternalInput")
    wqk1 = nc.dram_tensor("wqk1", [128, KT_TILES, 2 * 128], BF16,
                          kind="ExternalInput")
    # wv pre-augmented (64 cols + zero col per head) and pre-shuffled to
    # [p, ko, 260]
    wv = nc.dram_tensor("wv", [128, KT_TILES, VAUG], BF16,
                        kind="ExternalInput")
    # per-partition bias cols: bq_m0, bq_m1, bk_m0, bk_m1
    bqk = nc.dram_tensor("bqk", [128, 4], F32, kind="ExternalInput")
    # bv (+1.0 at each head's ones column), host-replicated to all
    # partitions so DVE can apply it elementwise during V evacuation
    bv_aug = nc.dram_tensor("bv_aug", [128, VAUG], BF16, kind="ExternalInput")
    out_raw = nc.dram_tensor("out_raw", [VAUG, S], F32, kind="ExternalOutput")

    with tile.TileContext(nc) as tc:
        with (
            tc.tile_pool(name="consts", bufs=1) as consts,
            tc.tile_pool(name="work", bufs=4) as work,
            tc.tile_pool(name="outp", bufs=4) as outp,
            # scores [128,1024] double-buffered: 4 banks
            tc.tile_pool(name="ps2", bufs=2, space="PSUM") as ps2,
            # ctx accumulators [65,512] x2 per pass: 2 banks
            tc.tile_pool(name="psA", bufs=2, space="PSUM") as psA,
            # V-projection scratch: 1 bank
            tc.tile_pool(name="psV", bufs=1, space="PSUM") as psV,
            # QK-projection chunk accumulator: 1 bank
            tc.tile_pool(name="psK", bufs=1, space="PSUM") as psK,
        ):
            # ---- PE warmup: ~4us of dummy matmuls so the HAM clock
            # gate reaches 8/8 before the real projections start ----
            wu = consts.tile([128, 128], BF16)
            nc.vector.memset(wu[:], 0)
            wups = psV.tile([128, QC], F32, tag="v", name="warmup")
            N_WARM = 60
            for i in range(N_WARM):
                c = (i % 4) * 128
                nc.tensor.matmul(
                    wups[:, c:c + 128], wu[:], wu[:],
                    start=(i < 4), stop=(i >= N_WARM - 4),
                )

            # ---- input DMAs: xT split into (sc, ko) pieces, SEQ-major so
            # attention over early q/k blocks can start while the rest of
            # x streams in; triggers spread over sync/scalar/gpsimd ----
            wqk_sb = consts.tile([128, 2, KT_TILES, 2 * 128], BF16)
            xT_sb = consts.tile([128, KT_TILES, S], BF16)
            xT_r = xT.rearrange("(ko p) s -> p ko s", p=128)
            wv_sb = consts.tile([128, KT_TILES, VAUG], BF16)
            bqk_sb = consts.tile([128, 4], F32)
            bvaug_sb = consts.tile([128, VAUG], BF16)

            def xt_piece(sc, ko):
                sl = (slice(None), ko, slice(sc * QC, (sc + 1) * QC))
                return (xT_sb[sl], xT_r[sl])

            # NOTE: never put input triggers on the scalar queue — DMA
            # ring flow-control makes late triggers wait on earlier
            # transfers, and the exp ACTIVATEs would queue behind them.
            queues = {"sync": [], "gpsimd": []}
            qnames = ["sync", "gpsimd"]
            h = KT_TILES // 2
            queues["sync"].append((wqk_sb[:, 0, 0:h], wqk0[:, 0:h]))
            queues["gpsimd"].append((wqk_sb[:, 0, h:], wqk0[:, h:]))
            for i, (sc, ko) in enumerate(
                [(sc, ko) for sc in range(N_QC) for ko in range(KT_TILES)]
            ):
                queues[qnames[i % 2]].append(xt_piece(sc, ko))
                if sc == 0 and ko == 5:
                    queues["gpsimd"].append((wv_sb[:], wv[:]))
                    queues["sync"].append((bvaug_sb[:], bv_aug[:]))
                    queues["sync"].append((bqk_sb[:], bqk[:]))
                if sc == 1 and ko == 7:
                    queues["gpsimd"].append((wqk_sb[:, 1], wqk1[:]))
            for qn, lst in queues.items():
                eng = getattr(nc, qn)
                for dst, sr in lst:
                    eng.dma_start(dst, sr)

            QT_sb = consts.tile([128, 2, S], BF16)
            KT_sb = consts.tile([128, 2, S], BF16)
            v_sb = consts.tile([128, ST_TILES, VAUG], BF16)

            def w_slice(w, m, ko):
                return wqk_sb[:, m, ko, w * 128:(w + 1) * 128]

            def qk_evac(dst, ps, w, m, sc):
                # bias[d'] is per-partition: DVE adds it during evacuation
                nc.vector.tensor_scalar_add(
                    dst[:, m, sc * QC:(sc + 1) * QC],
                    ps,
                    bqk_sb[:, w * 2 + m:w * 2 + m + 1],
                )

            # QK projection as chunks of 8 contraction matmuls; drained
            # 2 per attention kt-iteration against per-pass deadlines
            def make_chunk(w, m, sc, pool=None):
                state = {}
                pl = pool if pool is not None else psK
                ptag = "v" if pool is not None else "k"

                def mk(ko):
                    def thunk():
                        if ko == 0:
                            state["ps"] = pl.tile(
                                [128, QC], F32, tag=ptag, name=f"pj{w}{m}{sc}",
                            )
                        nc.tensor.matmul(
                            state["ps"][:], w_slice(w, m, ko),
                            xT_sb[:, ko, sc * QC:(sc + 1) * QC],
                            start=(ko == 0), stop=(ko == KT_TILES - 1),
                        )
                        if ko == KT_TILES - 1:
                            qk_evac(
                                QT_sb if w == 0 else KT_sb,
                                state["ps"][:], w, m, sc,
                            )
                    return thunk
                return [mk(ko) for ko in range(KT_TILES)]

            Q, K = 0, 1
            # per-pass proj thunk queues (2 drained per kt):
            #   lead: Q/K(m0,sc0)   (before pass 0)
            #   pass0: K(m0,sc1..3) just ahead of the kt that needs them,
            #          then Q(m0,sc1); v-projection also rides here
            #   pass1: Q(m0,sc2), Q(m0,sc3), Q(m1,sc0), K(m1,sc0)
            #   pass2: Q/K(m1,sc1), Q/K(m1,sc2)
            #   pass3: Q(m1,sc3), K(m1,sc3)
            lead_chunks = make_chunk(Q, 0, 0) + make_chunk(K, 0, 0)
            pass_proj = {
                0: (make_chunk(K, 0, 1) + make_chunk(K, 0, 2)
                    + make_chunk(K, 0, 3) + make_chunk(Q, 0, 1)),
                1: (make_chunk(Q, 0, 2) + make_chunk(Q, 0, 3, psV)
                    + make_chunk(Q, 1, 0) + make_chunk(K, 1, 0, psV)),
                2: (make_chunk(Q, 1, 1) + make_chunk(K, 1, 1, psV)
                    + make_chunk(Q, 1, 2) + make_chunk(K, 1, 2, psV)),
                3: make_chunk(Q, 1, 3) + make_chunk(K, 1, 3, psV),
            }
            # drain the lead with dummy filler matmuls between real ones:
            # the real MMs are DMA-gated, the fillers keep the PE's HAM
            # activity window busy so the clock stays at 8/8
            for i, t in enumerate(lead_chunks):
                t()
                for j in range(2):
                    c = ((2 * i + j) % 4) * 128
                    nc.tensor.matmul(
                        wups[:, c:c + 128], wu[:], wu[:],
                        start=True, stop=True,
                    )

            def v_proj(st):
                psv = psV.tile([128, QC], F32, tag="v", name="ps_v")
                for ko in range(KT_TILES):
                    nc.tensor.matmul(
                        psv[:, 0:VAUG],
                        xT_sb[:, ko, st * 128:(st + 1) * 128],
                        wv_sb[:, ko, :],
                        start=(ko == 0), stop=(ko == KT_TILES - 1),
                    )
                # bias + per-head ones columns land during evacuation
                nc.vector.tensor_add(
                    v_sb[:, st, :], psv[:, 0:VAUG], bvaug_sb[:]
                )

            def attn_pass(pi, p, qc):
                proj = pass_proj.get(pi, [])
                ctxs = [
                    psA.tile([65, QC], F32, tag="sA", name=f"ctx{hh}")
                    for hh in range(2)
                ]

                def ctx_mms(es, kt):
                    for hh in range(2):
                        h = 2 * p + hh
                        nc.tensor.matmul(
                            ctxs[hh][:],
                            v_sb[:, kt, h * 65:(h + 1) * 65],
                            es[:, hh * QC:(hh + 1) * QC],
                            start=(kt == 0), stop=(kt == ST_TILES - 1),
                        )

                prev = None
                for kt in range(ST_TILES):
                    if pi == 0:
                        v_proj(kt)  # ctx at kt consumes exactly V tile kt
                    for t in proj[kt * 2:kt * 2 + 2]:
                        t()
                    ssc = ps2.tile([128, 1024], F32, tag="s2", name="ssc")
                    for hh in range(2):  # row-packed pair, runs concurrent
                        rows = slice(hh * 64, hh * 64 + 64)
                        nc.tensor.matmul(
                            ssc[:, hh * QC:(hh + 1) * QC],
                            KT_sb[rows, p, kt * 128:(kt + 1) * 128],
                            QT_sb[rows, p, qc * QC:(qc + 1) * QC],
                            start=True, stop=True,
                        )
                    es = work.tile([128, 1024], BF16, tag="es", name="es")
                    nc.scalar.activation(
                        es[:], ssc[:],
                        mybir.ActivationFunctionType.Exp,
                        scale=0.125,
                    )
                    if prev is not None:
                        ctx_mms(*prev)  # lag 1 kt: never stall on fresh exp
                    prev = (es, kt)
                ctx_mms(*prev)
                for hh in range(2):
                    h = 2 * p + hh
                    ctx_sb = outp.tile([65, QC], F32, tag="o", name="ctx_sb")
                    nc.vector.tensor_copy(out=ctx_sb[:], in_=ctxs[hh][:])
                    nc.sync.dma_start(
                        out_raw[h * 65:(h + 1) * 65, qc * QC:(qc + 1) * QC],
                        ctx_sb[:],
                    )

            for pi, (p, qc) in enumerate(
                [(0, 0), (0, 1), (0, 2), (0, 3),
                 (1, 0), (1, 1), (1, 2), (1, 3)]
            ):
                attn_pass(pi, p, qc)
    nc.compile()
    return nc


_NC_CACHE = None


def _get_nc():
    global _NC_CACHE
    if _NC_CACHE is None:
        _NC_CACHE = _build_kernel()
    return _NC_CACHE


def _prep_core_inputs(hidden_states, Wq, bq, Wk, bk, Wv, bv):
    """Host-side sharding: returns list of 8 in_maps (bf16 pre-cast)."""
    xTs = [
        np.ascontiguousarray(hidden_states[b].T).astype(NP_BF16)
        for b in range(B)
    ]
    in_maps = []
    for c in range(N_CORES):
        b, g = divmod(c, GROUPS)
        cs = slice(g * DG, (g + 1) * DG)
        wq_g, wk_g, wv_g = Wq[:, cs], Wk[:, cs], Wv[:, cs]
        bq_g, bk_g, bv_g = bq[cs], bk[cs], bv[cs]

        # per m: [p, ko, wq 128 | wk 128] (SBUF layout, contiguous)
        wq_r = wq_g.reshape(KT_TILES, 128, DG).transpose(1, 0, 2)
        wk_r = wk_g.reshape(KT_TILES, 128, DG).transpose(1, 0, 2)
        wqk_ms = [
            np.concatenate(
                [wq_r[:, :, m * 128:(m + 1) * 128],
                 wk_r[:, :, m * 128:(m + 1) * 128]], axis=2,
            )
            for m in range(2)
        ]

        wv_aug = np.zeros((HIDDEN, VAUG), dtype=np.float32)
        bv_aug = np.zeros((1, VAUG), dtype=np.float32)  # replicated below
        for h in range(HG):
            wv_aug[:, h * 65:h * 65 + 64] = wv_g[:, h * 64:(h + 1) * 64]
            bv_aug[0, h * 65:h * 65 + 64] = bv_g[h * 64:(h + 1) * 64]
            bv_aug[0, h * 65 + 64] = 1.0

        bqk = np.stack(
            [bq_g[:128], bq_g[128:], bk_g[:128], bk_g[128:]], axis=1
        ).astype(np.float32)

        wv_r = wv_aug.reshape(KT_TILES, 128, VAUG).transpose(1, 0, 2)
        in_maps.append(
            {
                "xT": xTs[b],
                "wqk0": np.ascontiguousarray(wqk_ms[0]).astype(NP_BF16),
                "wqk1": np.ascontiguousarray(wqk_ms[1]).astype(NP_BF16),
                "wv": np.ascontiguousarray(wv_r).astype(NP_BF16),
                "bqk": np.ascontiguousarray(bqk),
                "bv_aug": np.broadcast_to(
                    bv_aug.astype(NP_BF16), (128, VAUG)
                ).copy(),
            }
        )
    return in_maps


def _unshard(results):
    out = np.empty((B, S, HIDDEN), dtype=np.float32)
    for c in range(N_CORES):
        b, g = divmod(c, GROUPS)
        raw = results[c]["out_raw"]  # [260, 2048]
        for h in range(HG):
            ctx = raw[h * 65:h * 65 + 64]          # [64, S]
            sums = raw[h * 65 + 64]                # [S]
            col0 = g * DG + h * HEAD
            out[b, :, col0:col0 + HEAD] = (ctx / sums).T
    return out


def kernel(**inputs):
    inputs = {k: np.asarray(v, dtype=np.float32) for k, v in inputs.items()}
    nc = _get_nc()
    in_maps = _prep_core_inputs(**inputs)
    res = run_bass_kernel_spmd(nc, in_maps, core_ids=list(range(N_CORES)))
    return _unshard(res.results)


if __name__ == "__main__":
    rng = np.random.default_rng(0)
    scale = 1.0 / np.sqrt(HIDDEN)
    ins = {
        "hidden_states": rng.standard_normal((B, S, HIDDEN), dtype=np.float32),
        "Wq": rng.standard_normal((HIDDEN, HIDDEN), dtype=np.float32) * scale,
        "bq": rng.standard_normal(HIDDEN, dtype=np.float32) * 0.01,
        "Wk": rng.standard_normal((HIDDEN, HIDDEN), dtype=np.float32) * scale,
        "bk": rng.standard_normal(HIDDEN, dtype=np.float32) * 0.01,
        "Wv": rng.standard_normal((HIDDEN, HIDDEN), dtype=np.float32) * scale,
        "bv": rng.standard_normal(HIDDEN, dtype=np.float32) * 0.01,
    }
    out = kernel(**ins)

    def ref(x, Wq, bq, Wk, bk, Wv, bv):
        q = (x @ Wq + bq).reshape(B, S, NUM_HEADS, HEAD).transpose(0, 2, 1, 3)
        k = (x @ Wk + bk).reshape(B, S, NUM_HEADS, HEAD).transpose(0, 2, 1, 3)
        v = (x @ Wv + bv).reshape(B, S, NUM_HEADS, HEAD).transpose(0, 2, 1, 3)
        s = np.einsum("bhqd,bhkd->bhqk", q, k) / np.sqrt(HEAD)
        s = s - s.max(-1, keepdims=True)
        p = np.exp(s)
        p /= p.sum(-1, keepdims=True)
        c = np.einsum("bhqk,bhkd->bhqd", p, v)
        return c.transpose(0, 2, 1, 3).reshape(B, S, HIDDEN)

    exp = ref(
        ins["hidden_states"].astype(np.float64),
        ins["Wq"].astype(np.float64), ins["bq"].astype(np.float64),
        ins["Wk"].astype(np.float64), ins["bk"].astype(np.float64),
        ins["Wv"].astype(np.float64), ins["bv"].astype(np.float64),
    )
    print("L2 rel err:", np.linalg.norm(out - exp) / np.linalg.norm(exp))
    print("max abs err:", np.abs(out - exp).max())
